# revision 30
# baseline (speedup 1.0000x reference)
"""Bipartite GNN message passing on 8 Trainium2 NeuronCores.

Math reformulation: relu(h[idx] @ W + b) == relu(h @ W + b)[idx], so each
direction-layer is: per-node message MLP (z) -> gather z rows by edge ->
segment-sum -> update MLP.  Sharding: aggregation-side nodes are split into
8 contiguous ranges (one per core); each core owns ALL edges targeting its
range, so it computes complete aggregates locally (no AllReduce).  Only the
small per-shard z tensors are AllGathered (fp16) each direction-layer, in
4 (src) / 2 (dst) pipelined pieces launched mid-sweep as soon as the h rows
they need are updated, so gathers start before the full AllGather lands.

Segment-sum on the tensor engine: edges sorted by (window512, table,
sub128); for each 128-edge chunk a one-hot S[e,j] = (dst_local[e] == j)
matrix (DVE is_equal against an iota tile) turns the segment sum into
psum[64f, 128dst] += gathered_z[128e, 64f].T @ S[128e, 128dst], accumulated
in a PSUM bank per 512-dst window (start=True clears, no memset) and
flushed once per window into a fused update-MLP (2 accumulating matmuls +
relu activation straight back into h).

Gathers use dma_gather (SWDGE): int16 indices into <=25088-row table slices
of the AllGathered z pieces, 256B padded fp16 rows.  Descriptor generation
runs on a Q7 core pair selected by queue_num, so gathers are issued
round-robin on queues 0-3 with 6 gather buffers to keep 4 desc-gens in
flight (this was the baseline bottleneck: ~7.8ns/row on one pair).

SPMD: one NEFF for all 8 cores; the chunk schedule is the max over cores;
cores pad with (idx=0, dst_local=sentinel) edges that contribute zero.
"""
import numpy as np

D = 64
CHUNK = 128
SUB = 128
WIN = 512
SENT = 999.0
N_CORES = 8
GCAP = 4608  # max slots per dma_gather
SB = 32      # chunks per S-build batch
NQ = 4       # SWDGE queues


class Cfg:
    def __init__(self, ns_pad, nd_pad):
        self.NS_PAD, self.ND_PAD = ns_pad, nd_pad
        self.SRC_SH, self.DST_SH = ns_pad // N_CORES, nd_pad // N_CORES
        # gather-side piece/table geometry: src z is AllGathered in 4 pieces
        # (piece == table), dst z in 2 pieces (piece == table)
        self.S_Q = self.SRC_SH // 4              # rows/core per AG piece (src)
        self.D_HALF = self.DST_SH // 2           # rows/core per AG piece (dst)
        self.S_TAB = self.S_Q * N_CORES          # rows per src piece/table
        self.D_TAB = self.D_HALF * N_CORES       # rows per dst piece/table
        assert self.S_TAB <= 32767 and self.D_TAB <= 32767
        assert self.SRC_SH % CHUNK == 0 and self.DST_SH % CHUNK == 0


REAL_CFG = Cfg(100352, 50176)


def _src_table_map(cfg, g):
    """src node g -> (table q in 0..3, row in table).  piece p == q."""
    c = g // cfg.SRC_SH
    off = g % cfg.SRC_SH
    q = off // cfg.S_Q
    row = c * cfg.S_Q + (off % cfg.S_Q)
    return q, row


def _dst_table_map(cfg, g):
    """dst node g -> (table q in 0..1, row in table).  piece p == q."""
    c = g // cfg.DST_SH
    off = g % cfg.DST_SH
    p = off // cfg.D_HALF
    row = c * cfg.D_HALF + (off % cfg.D_HALF)
    return p, row


def _build_plan(cfg, gather_idx, seg_idx, table_map, n_tab, shard):
    """SPMD-uniform edge plan for one direction.

    Slot order: (window512, table q, sub128, chunk).  Returns per-core idx16
    [128, TOT/16] int16 and dw [128, TOT/128] fp16, plus uniform
    windows [(w, [(q, si, k)])] and gathers [(q, slot0, nsl)].
    """
    n_w = (shard + WIN - 1) // WIN
    n_si_tot = shard // SUB
    q_all, row_all = table_map(cfg, gather_idx)
    core_of = seg_idx // shard
    per_core = []
    nsi_of_w = [min(WIN // SUB, n_si_tot - w * (WIN // SUB))
                for w in range(n_w)]
    Kmax = np.zeros((n_w, n_tab, WIN // SUB), np.int64)
    for c in range(N_CORES):
        m = core_of == c
        row = row_all[m]
        q = q_all[m]
        s = seg_idx[m] - c * shard
        w = s // WIN
        si = (s % WIN) // SUB
        key = (w * n_tab + q) * (WIN // SUB) + si
        order = np.argsort(key, kind="stable")
        row, q, s, w, si = row[order], q[order], s[order], w[order], si[order]
        per_core.append((row, s % SUB, w, q, si))
        cnt = np.zeros((n_w, n_tab, WIN // SUB), np.int64)
        np.add.at(cnt, (w, q, si), 1)
        Kmax = np.maximum(Kmax, (cnt + CHUNK - 1) // CHUNK)

    # every (w, si) needs >=1 chunk so the PSUM column range is written
    for w in range(n_w):
        for si in range(nsi_of_w[w]):
            if Kmax[w, :, si].sum() == 0:
                Kmax[w, 0, si] = 1

    slots_per_group = Kmax * CHUNK
    starts = np.zeros_like(slots_per_group)
    total = 0
    for w in range(n_w):
        for q in range(n_tab):
            for si in range(nsi_of_w[w]):
                starts[w, q, si] = total
                total += int(slots_per_group[w, q, si])
    assert total % CHUNK == 0

    idx16_list, dw_list = [], []
    for c in range(N_CORES):
        row, dwv, w, q, si = per_core[c]
        G = np.zeros(total, np.int32)
        DW = np.full(total, SENT, np.float32)
        kk = (w * n_tab + q) * (WIN // SUB) + si
        bounds = np.flatnonzero(np.diff(kk)) + 1
        for grp in np.split(np.arange(len(row)), bounds):
            if len(grp) == 0:
                continue
            ww, qq, sg = int(w[grp[0]]), int(q[grp[0]]), int(si[grp[0]])
            st = int(starts[ww, qq, sg])
            n = len(grp)
            G[st:st + n] = row[grp]
            DW[st:st + n] = dwv[grp]
        i16 = np.empty((128, total // 16), np.int16)
        base = G.astype(np.int16).reshape(total // 16, 16).T
        for k in range(8):
            i16[16 * k:16 * (k + 1)] = base
        dw = DW.astype(np.float16).reshape(total // CHUNK, CHUNK).T
        idx16_list.append(i16)
        dw_list.append(np.ascontiguousarray(dw))

    windows = []
    for w in range(n_w):
        groups = []
        for q in range(n_tab):
            for si in range(nsi_of_w[w]):
                k = int(Kmax[w, q, si])
                if k:
                    groups.append((q, si, k))
        windows.append((w, groups))

    gathers = []
    for w in range(n_w):
        for q in range(n_tab):
            lo = int(starts[w, q, 0])
            hi = lo + int(slots_per_group[w, q, :nsi_of_w[w]].sum())
            p = lo
            while p < hi:
                n = min(GCAP, hi - p)
                gathers.append((q, p, n))
                p += n
    return dict(idx16=idx16_list, dw=dw_list, windows=windows, gathers=gathers,
                total=total)


def _host_prep(cfg, inputs):
    f32 = np.float32
    x_src = np.asarray(inputs["x_src"], f32)
    x_dst = np.asarray(inputs["x_dst"], f32)
    src_idx = np.asarray(inputs["src_idx"]).astype(np.int64)
    dst_idx = np.asarray(inputs["dst_idx"]).astype(np.int64)
    L = np.asarray(inputs["W_msg_sd"]).shape[0]

    ns, nd = x_src.shape[0], x_dst.shape[0]
    xs = np.zeros((cfg.NS_PAD, D), f32)
    xs[:ns] = x_src
    xd = np.zeros((cfg.ND_PAD, D), f32)
    xd[:nd] = x_dst

    plan_sd = _build_plan(cfg, src_idx, dst_idx, _src_table_map, 4, cfg.DST_SH)
    plan_ds = _build_plan(cfg, dst_idx, src_idx, _dst_table_map, 2, cfg.SRC_SH)

    def stack_wb(wk, bk):
        w = np.asarray(inputs[wk], f32)
        b = np.asarray(inputs[bk], f32)
        out = np.empty((L, D + 1, D), np.float16)
        out[:, :D] = w.astype(np.float16)
        out[:, D] = b.astype(np.float16)
        return out

    host = dict(
        L=L,
        Wbm_sd=stack_wb("W_msg_sd", "b_msg_sd"),
        Wbm_ds=stack_wb("W_msg_ds", "b_msg_ds"),
        Wu_dst=np.asarray(inputs["W_upd_dst"], f32).astype(np.float16),
        Wu_src=np.asarray(inputs["W_upd_src"], f32).astype(np.float16),
        bu_dst=np.asarray(inputs["b_upd_dst"], f32)[:, :, None],
        bu_src=np.asarray(inputs["b_upd_src"], f32)[:, :, None],
        Win_src=np.asarray(inputs["W_in_src"], f32),
        Win_dst=np.asarray(inputs["W_in_dst"], f32),
        bin_src=np.asarray(inputs["b_in_src"], f32)[:, None],
        bin_dst=np.asarray(inputs["b_in_dst"], f32)[:, None],
        iota=np.tile(np.arange(WIN, dtype=np.float16), (128, 1)),
        xsT=[np.ascontiguousarray(xs[c * cfg.SRC_SH:(c + 1) * cfg.SRC_SH].T)
             for c in range(N_CORES)],
        xdT=[np.ascontiguousarray(xd[c * cfg.DST_SH:(c + 1) * cfg.DST_SH].T)
             for c in range(N_CORES)],
        plan_sd=plan_sd, plan_ds=plan_ds,
    )
    return host


def _build_nc(cfg, host):
    import concourse.bass as bass
    import concourse.tile as tile
    from concourse import bacc, mybir

    dt = mybir.dt
    L = host["L"]
    plan_sd, plan_ds = host["plan_sd"], host["plan_ds"]
    TOT_SD, TOT_DS = plan_sd["total"], plan_ds["total"]

    nc = bacc.Bacc("TRN2", target_bir_lowering=False, debug=False,
                   num_devices=N_CORES, num_swdge_queues=NQ)

    def inp(name, shape, dtype):
        return nc.dram_tensor(name, shape, dtype, kind="ExternalInput").ap()

    xT_src = inp("xT_src", [D, cfg.SRC_SH], dt.float32)
    xT_dst = inp("xT_dst", [D, cfg.DST_SH], dt.float32)
    Win_src = inp("Win_src", [D, D], dt.float32)
    Win_dst = inp("Win_dst", [D, D], dt.float32)
    bin_src = inp("bin_src", [D, 1], dt.float32)
    bin_dst = inp("bin_dst", [D, 1], dt.float32)
    Wbm_sd = inp("Wbm_sd", [L, D + 1, D], dt.float16)
    Wbm_ds = inp("Wbm_ds", [L, D + 1, D], dt.float16)
    Wu_dst = inp("Wu_dst", [L, 2 * D, D], dt.float16)
    Wu_src = inp("Wu_src", [L, 2 * D, D], dt.float16)
    bu_dst = inp("bu_dst", [L, D, 1], dt.float32)
    bu_src = inp("bu_src", [L, D, 1], dt.float32)
    iota_in = inp("iota", [128, WIN], dt.float16)
    idx_sd = inp("idx_sd", [128, TOT_SD // 16], dt.int16)
    dw_sd = inp("dw_sd", [128, TOT_SD // CHUNK], dt.float16)
    idx_ds = inp("idx_ds", [128, TOT_DS // 16], dt.int16)
    dw_ds = inp("dw_ds", [128, TOT_DS // CHUNK], dt.float16)
    out_hd = nc.dram_tensor("out_hd", [D, cfg.DST_SH], dt.float16,
                            kind="ExternalOutput").ap()

    # per-piece z shard (local) and AllGathered piece tensors (shared)
    zs_sh = [nc.dram_tensor(f"zs_sh{p}", [cfg.S_Q, 128], dt.float16).ap()
             for p in range(4)]
    zd_sh = [nc.dram_tensor(f"zd_sh{p}", [cfg.D_HALF, 128], dt.float16).ap()
             for p in range(2)]
    zs_pc = [nc.dram_tensor(f"zs_pc{p}", [cfg.S_TAB, 128], dt.float16,
                            addr_space="Shared").ap() for p in range(4)]
    zd_pc = [nc.dram_tensor(f"zd_pc{p}", [cfg.D_TAB, 128], dt.float16,
                            addr_space="Shared").ap() for p in range(2)]

    RELU = mybir.ActivationFunctionType.Relu
    EQ = mybir.AluOpType.is_equal
    rg = [list(range(N_CORES))]

    with tile.TileContext(nc) as tc:
        from contextlib import ExitStack
        with ExitStack() as ctx:
            pers = ctx.enter_context(tc.tile_pool(name="pers", bufs=1))
            ps_agg = ctx.enter_context(
                tc.tile_pool(name="psagg", bufs=4, space="PSUM"))
            ps_mlp = ctx.enter_context(
                tc.tile_pool(name="psmlp", bufs=3, space="PSUM"))
            gath = ctx.enter_context(tc.tile_pool(name="gath", bufs=10))
            idxg = ctx.enter_context(tc.tile_pool(name="idxg", bufs=16))
            spool = ctx.enter_context(tc.tile_pool(name="spool", bufs=2))
            work = ctx.enter_context(tc.tile_pool(name="work", bufs=4))

            h_s = pers.tile([D + 1, cfg.SRC_SH], dt.float16, name="h_s")
            h_d = pers.tile([D + 1, cfg.DST_SH], dt.float16, name="h_d")
            iota_t = pers.tile([128, WIN], dt.float16)
            dw_sd_t = pers.tile([128, TOT_SD // CHUNK], dt.float16)
            dw_ds_t = pers.tile([128, TOT_DS // CHUNK], dt.float16)

            nc.sync.dma_start(out=iota_t[:], in_=iota_in[:])
            nc.sync.dma_start(out=dw_sd_t[:], in_=dw_sd[:])
            nc.sync.dma_start(out=dw_ds_t[:], in_=dw_ds[:])

            w_enc_s = pers.tile([D, D], dt.float32)
            w_enc_d = pers.tile([D, D], dt.float32)
            b_enc_s = pers.tile([D, 1], dt.float32)
            b_enc_d = pers.tile([D, 1], dt.float32)
            nc.sync.dma_start(out=w_enc_s[:], in_=Win_src[:])
            nc.sync.dma_start(out=w_enc_d[:], in_=Win_dst[:])
            nc.sync.dma_start(out=b_enc_s[:], in_=bin_src[:])
            nc.sync.dma_start(out=b_enc_d[:], in_=bin_dst[:])

            wbm_t, wu_t, bu_t = {}, {}, {}
            for l in range(L):
                for key, src in (("sd", Wbm_sd), ("ds", Wbm_ds)):
                    t = pers.tile([D + 1, D], dt.float16, name=f"wbm_{key}{l}")
                    nc.sync.dma_start(out=t[:], in_=src[l])
                    wbm_t[key, l] = t
                for key, src in (("dst", Wu_dst), ("src", Wu_src)):
                    th = pers.tile([D, D], dt.float16, name=f"wuh_{key}{l}")
                    ta = pers.tile([D, D], dt.float16, name=f"wua_{key}{l}")
                    nc.sync.dma_start(out=th[:], in_=src[l, 0:D, :])
                    nc.sync.dma_start(out=ta[:], in_=src[l, D:2 * D, :])
                    wu_t[key, l] = (th, ta)
                for key, src in (("dst", bu_dst), ("src", bu_src)):
                    t = pers.tile([D, 1], dt.float32, name=f"bu_{key}{l}")
                    nc.sync.dma_start(out=t[:], in_=src[l])
                    bu_t[key, l] = t

            for t in (h_s, h_d):
                nc.vector.memset(t[D:D + 1, :], 1.0)

            # one-time zero fill of z-shard pad columns (never written later)
            zeros64 = pers.tile([128, D], dt.float16, name="zeros64")
            nc.vector.memset(zeros64[:], 0.0)
            for z_list, half in ((zs_sh, cfg.S_Q), (zd_sh, cfg.D_HALF)):
                for z in z_list:
                    for k in range(0, half, CHUNK):
                        cw = min(CHUNK, half - k)
                        nc.sync.dma_start(out=z[k:k + cw, D:128],
                                          in_=zeros64[:cw, :])

            def encoder(xT, w_t, b_t, h_out, n, hooks=None):
                for j0 in range(0, n, WIN):
                    w = min(WIN, n - j0)
                    xs = work.tile([D, WIN], dt.float32, tag="xs")
                    nc.sync.dma_start(out=xs[:, :w], in_=xT[:, j0:j0 + w])
                    ps = ps_mlp.tile([D, WIN], dt.float32, tag="mlp")
                    nc.tensor.matmul(out=ps[:, :w], lhsT=w_t[:], rhs=xs[:, :w],
                                     start=True, stop=True)
                    nc.scalar.activation(out=h_out[0:D, j0:j0 + w],
                                         in_=ps[:, :w], func=RELU, bias=b_t[:])
                    if hooks and j0 // WIN in hooks:
                        hooks[j0 // WIN]()

            def z_piece(h_in, wbm, z_sh, half, p):
                r0 = p * half
                for k in range(0, half, CHUNK):
                    cw = min(CHUNK, half - k)
                    ps = ps_mlp.tile([CHUNK, D], dt.float32, tag="mlp")
                    nc.tensor.matmul(
                        out=ps[:cw, :],
                        lhsT=h_in[0:D + 1, r0 + k:r0 + k + cw],
                        rhs=wbm[:], start=True, stop=True)
                    zs = work.tile([CHUNK, D], dt.float16, tag="zstage")
                    nc.vector.tensor_scalar_max(out=zs[:cw, :], in0=ps[:cw, :],
                                                scalar1=0.0)
                    nc.sync.dma_start(out=z_sh[k:k + cw, 0:D], in_=zs[:cw, :])



            def z_ag(h_in, wbm, z_sh, z_pc, half, p):
                z_piece(h_in, wbm, z_sh[p], half, p)
                nc.gpsimd.collective_compute(
                    "AllGather", mybir.AluOpType.bypass, replica_groups=rg,
                    ins=[z_sh[p].opt()], outs=[z_pc[p].opt()])

            def sweep(plan, pieces, tab_of, idx_dram, dw_t, h_io, wu, bu,
                      shard, last=False, hooks=None):
                gathers = plan["gathers"]
                gtiles = []
                for g, (q, s0, nsl) in enumerate(gathers):
                    it = idxg.tile([128, nsl // 16], dt.int16, tag="idxg")
                    nc.sync.dma_start(
                        out=it[:], in_=idx_dram[:, s0 // 16:(s0 + nsl) // 16])
                    gt = gath.tile([128, nsl], dt.float16, tag="gt")
                    nc.gpsimd.dma_gather(
                        gt[:].rearrange("p (b e) -> p b e", e=128),
                        tab_of(q), it[:], nsl, nsl, 128, single_packet=False,
                        queue_num=g % NQ)
                    gtiles.append((s0, nsl, gt))

                gi = 0
                c = 0  # global chunk cursor
                s_tile = None
                for (w, groups) in plan["windows"]:
                    nvis = sum(k for (_, _, k) in groups)
                    w0 = w * WIN
                    ww = min(WIN, shard - w0)
                    ps = ps_agg.tile([D, WIN], dt.float32, tag="agg")
                    done = 0
                    for (q, si, kk) in groups:
                        for _ in range(kk):
                            if c % SB == 0:
                                nb = min(SB, dw_t.shape[1] - c)
                                s_tile = spool.tile([128, nb * SUB],
                                                    dt.float16, tag="s")
                                nc.vector.tensor_tensor(
                                    out=s_tile[:].rearrange(
                                        "p (b e) -> p b e", e=SUB),
                                    in0=dw_t[:, c:c + nb, None].to_broadcast(
                                        [128, nb, SUB]),
                                    in1=iota_t[:, 0:SUB][:, None, :]
                                    .to_broadcast([128, nb, SUB]),
                                    op=EQ)
                            s0, nsl, gt = gtiles[gi]
                            if c * CHUNK >= s0 + nsl:
                                gi += 1
                                s0, nsl, gt = gtiles[gi]
                            blk = (c * CHUNK - s0) // CHUNK
                            g3 = gt[:].rearrange("p (b e) -> p b e", e=128)
                            nc.tensor.matmul(
                                out=ps[:, si * SUB:(si + 1) * SUB],
                                lhsT=g3[:, blk, 0:D],
                                rhs=s_tile[:].rearrange(
                                    "p (b e) -> p b e", e=SUB)[:, c % SB, :],
                                start=(done == 0), stop=(done == nvis - 1),
                                skip_group_check=True)
                            done += 1
                            c += 1
                    # fused update for this window
                    ag = work.tile([D, WIN], dt.float16, tag="aggstage")
                    nc.vector.tensor_copy(out=ag[:, :ww], in_=ps[:, :ww])
                    psu = ps_mlp.tile([D, WIN], dt.float32, tag="mlp")
                    nc.tensor.matmul(out=psu[:, :ww], lhsT=wu[0][:],
                                     rhs=h_io[0:D, w0:w0 + ww],
                                     start=True, stop=False,
                                     skip_group_check=True)
                    nc.tensor.matmul(out=psu[:, :ww], lhsT=wu[1][:],
                                     rhs=ag[:, :ww],
                                     start=False, stop=True,
                                     skip_group_check=True)
                    nc.scalar.activation(out=h_io[0:D, w0:w0 + ww],
                                         in_=psu[:, :ww], func=RELU,
                                         bias=bu[:])
                    if last:
                        nc.sync.dma_start(out=out_hd[:, w0:w0 + ww],
                                          in_=h_io[0:D, w0:w0 + ww])
                    if hooks and w in hooks:
                        hooks[w]()

            def sd_tab(q):
                return zs_pc[q]

            def ds_tab(q):
                return zd_pc[q]

            def hook_w(half, p):
                # last window whose update covers rows of piece p
                return ((p + 1) * half - 1) // WIN

            enc_hooks = {}
            for p in range(4):
                w = hook_w(cfg.S_Q, p)
                enc_hooks.setdefault(w, []).append(
                    (z_ag, (h_s, wbm_t["sd", 0], zs_sh, zs_pc, cfg.S_Q, p)))
            enc_hooks = {w: (lambda fs=fs: [f(*a) for (f, a) in fs])
                         for w, fs in enc_hooks.items()}
            encoder(xT_src, w_enc_s, b_enc_s, h_s, cfg.SRC_SH,
                    hooks=enc_hooks)
            encoder(xT_dst, w_enc_d, b_enc_d, h_d, cfg.DST_SH)
            for l in range(L):
                # during the sd sweep, launch ds z+AG pieces as soon as the
                # h_d rows they need are updated
                hooks = None
                if l < L - 1:
                    hooks = {}
                    for p in range(2):
                        w = hook_w(cfg.D_HALF, p)
                        hooks.setdefault(w, []).append(
                            (z_ag, (h_d, wbm_t["ds", l], zd_sh, zd_pc,
                                    cfg.D_HALF, p)))
                    hooks = {w: (lambda fs=fs: [f(*a) for (f, a) in fs])
                             for w, fs in hooks.items()}
                sweep(plan_sd, zs_pc, sd_tab, idx_sd, dw_sd_t, h_d,
                      wu_t["dst", l], bu_t["dst", l], cfg.DST_SH,
                      last=(l == L - 1), hooks=hooks)
                if l == L - 1:
                    break
                hooks = {}
                for p in range(4):
                    w = hook_w(cfg.S_Q, p)
                    hooks.setdefault(w, []).append(
                        (z_ag, (h_s, wbm_t["sd", l + 1], zs_sh, zs_pc,
                                cfg.S_Q, p)))
                hooks = {w: (lambda fs=fs: [f(*a) for (f, a) in fs])
                         for w, fs in hooks.items()}
                sweep(plan_ds, zd_pc, ds_tab, idx_ds, dw_ds_t, h_s,
                      wu_t["src", l], bu_t["src", l], cfg.SRC_SH,
                      hooks=hooks)

    nc.compile()
    return nc


def make_in_maps(cfg, host):
    shared = dict(
        Win_src=host["Win_src"], Win_dst=host["Win_dst"],
        bin_src=host["bin_src"], bin_dst=host["bin_dst"],
        Wbm_sd=host["Wbm_sd"], Wbm_ds=host["Wbm_ds"],
        Wu_dst=host["Wu_dst"], Wu_src=host["Wu_src"],
        bu_dst=host["bu_dst"], bu_src=host["bu_src"],
        iota=host["iota"],
    )
    maps = []
    for c in range(N_CORES):
        m = dict(shared)
        m["xT_src"] = host["xsT"][c]
        m["xT_dst"] = host["xdT"][c]
        m["idx_sd"] = host["plan_sd"]["idx16"][c]
        m["dw_sd"] = host["plan_sd"]["dw"][c]
        m["idx_ds"] = host["plan_ds"]["idx16"][c]
        m["dw_ds"] = host["plan_ds"]["dw"][c]
        maps.append(m)
    return maps


def kernel(**inputs) -> np.ndarray:
    cfg = REAL_CFG
    host = _host_prep(cfg, inputs)
    nc = _build_nc(cfg, host)
    from concourse.bass_utils import run_bass_kernel_spmd
    res = run_bass_kernel_spmd(nc, make_in_maps(cfg, host),
                               core_ids=list(range(N_CORES)))
    nd = np.asarray(inputs["x_dst"]).shape[0]
    out = np.concatenate([res.results[c]["out_hd"].T for c in range(N_CORES)],
                         axis=0)[:nd]
    return out.astype(np.float32)


# revision 36
# speedup vs baseline: 1.1698x; 1.1698x over previous
"""Bipartite GNN message passing on 8 Trainium2 NeuronCores.

Math reformulation: relu(h[idx] @ W + b) == relu(h @ W + b)[idx], so each
direction-layer is: per-node message MLP (z) -> gather z rows by edge ->
segment-sum -> update MLP.  Sharding: aggregation-side nodes are split into
8 contiguous ranges (one per core); each core owns ALL edges targeting its
range, so it computes complete aggregates locally (no AllReduce).  Only the
small per-shard z tensors are AllGathered (fp16) each direction-layer, in
4 (src) / 2 (dst) pipelined pieces launched mid-sweep as soon as the h rows
they need are updated, so gathers start before the full AllGather lands.

Segment-sum on the tensor engine: edges sorted by (window512, table,
sub128); for each 128-edge chunk a one-hot S[e,j] = (dst_local[e] == j)
matrix (DVE is_equal against an iota tile) turns the segment sum into
psum[64f, 128dst] += gathered_z[128e, 64f].T @ S[128e, 128dst], accumulated
in a PSUM bank per 512-dst window (start=True clears, no memset) and
flushed once per window into a fused update-MLP (2 accumulating matmuls +
relu activation straight back into h).

Gathers use dma_gather (SWDGE): int16 indices into <=25088-row table slices
of the AllGathered z pieces, 256B padded fp16 rows.  Descriptor generation
runs on a Q7 core pair selected by queue_num, so gathers are issued
round-robin on queues 0-3 with 6 gather buffers to keep 4 desc-gens in
flight (this was the baseline bottleneck: ~7.8ns/row on one pair).

SPMD: one NEFF for all 8 cores; the chunk schedule is the max over cores;
cores pad with (idx=0, dst_local=sentinel) edges that contribute zero.
"""
import numpy as np

D = 64
CHUNK = 128
SUB = 128
WIN = 512
SENT = 999.0
N_CORES = 8
GCAP = 4608  # max slots per dma_gather
SB = 8       # chunks per S-build batch
NQ = 4       # SWDGE queues


class Cfg:
    def __init__(self, ns_pad, nd_pad):
        self.NS_PAD, self.ND_PAD = ns_pad, nd_pad
        self.SRC_SH, self.DST_SH = ns_pad // N_CORES, nd_pad // N_CORES
        # gather-side piece/table geometry: src z is AllGathered in 2 pieces
        # (piece == 2 tables), dst z in 2 pieces (piece == table)
        self.S_HALF = self.SRC_SH // 2           # rows/core per AG piece (src)
        self.D_HALF = self.DST_SH // 2           # rows/core per AG piece (dst)
        self.S_PIECE = self.S_HALF * N_CORES     # rows per src piece tensor
        self.S_TAB = self.S_PIECE // 2           # src table = half piece
        self.D_TAB = self.D_HALF * N_CORES       # rows per dst piece/table
        assert self.S_TAB <= 32767 and self.D_TAB <= 32767
        assert self.SRC_SH % CHUNK == 0 and self.DST_SH % CHUNK == 0


REAL_CFG = Cfg(100352, 50176)


def _src_table_map(cfg, g):
    """src node g -> (table q in 0..3, row in table).  piece p = q//2."""
    c = g // cfg.SRC_SH
    off = g % cfg.SRC_SH
    p = off // cfg.S_HALF
    q = p * 2 + c // 4
    row = (c % 4) * cfg.S_HALF + (off % cfg.S_HALF)
    return q, row


def _dst_table_map(cfg, g):
    """dst node g -> (table q in 0..1, row in table).  piece p == q."""
    c = g // cfg.DST_SH
    off = g % cfg.DST_SH
    p = off // cfg.D_HALF
    row = c * cfg.D_HALF + (off % cfg.D_HALF)
    return p, row


def _build_plan(cfg, gather_idx, seg_idx, table_map, n_tab, shard):
    """SPMD-uniform edge plan for one direction.

    Slot order: (window512, table q, sub128, chunk).  Returns per-core idx16
    [128, TOT/16] int16 and dw [128, TOT/128] fp16, plus uniform
    windows [(w, [(q, si, k)])] and gathers [(q, slot0, nsl)].
    """
    n_w = (shard + WIN - 1) // WIN
    n_si_tot = shard // SUB
    q_all, row_all = table_map(cfg, gather_idx)
    core_of = seg_idx // shard
    per_core = []
    nsi_of_w = [min(WIN // SUB, n_si_tot - w * (WIN // SUB))
                for w in range(n_w)]
    Kmax = np.zeros((n_w, n_tab, WIN // SUB), np.int64)
    for c in range(N_CORES):
        m = core_of == c
        row = row_all[m]
        q = q_all[m]
        s = seg_idx[m] - c * shard
        w = s // WIN
        si = (s % WIN) // SUB
        key = (w * n_tab + q) * (WIN // SUB) + si
        order = np.argsort(key, kind="stable")
        row, q, s, w, si = row[order], q[order], s[order], w[order], si[order]
        per_core.append((row, s % SUB, w, q, si))
        cnt = np.zeros((n_w, n_tab, WIN // SUB), np.int64)
        np.add.at(cnt, (w, q, si), 1)
        Kmax = np.maximum(Kmax, (cnt + CHUNK - 1) // CHUNK)

    # every (w, si) needs >=1 chunk so the PSUM column range is written
    for w in range(n_w):
        for si in range(nsi_of_w[w]):
            if Kmax[w, :, si].sum() == 0:
                Kmax[w, 0, si] = 1

    slots_per_group = Kmax * CHUNK
    starts = np.zeros_like(slots_per_group)
    total = 0
    for w in range(n_w):
        for q in range(n_tab):
            for si in range(nsi_of_w[w]):
                starts[w, q, si] = total
                total += int(slots_per_group[w, q, si])
    assert total % CHUNK == 0

    idx16_list, dw_list = [], []
    for c in range(N_CORES):
        row, dwv, w, q, si = per_core[c]
        G = np.zeros(total, np.int32)
        DW = np.full(total, SENT, np.float32)
        kk = (w * n_tab + q) * (WIN // SUB) + si
        bounds = np.flatnonzero(np.diff(kk)) + 1
        for grp in np.split(np.arange(len(row)), bounds):
            if len(grp) == 0:
                continue
            ww, qq, sg = int(w[grp[0]]), int(q[grp[0]]), int(si[grp[0]])
            st = int(starts[ww, qq, sg])
            n = len(grp)
            G[st:st + n] = row[grp]
            DW[st:st + n] = dwv[grp]
        i16 = np.empty((128, total // 16), np.int16)
        base = G.astype(np.int16).reshape(total // 16, 16).T
        for k in range(8):
            i16[16 * k:16 * (k + 1)] = base
        dw = DW.astype(np.float16).reshape(total // CHUNK, CHUNK).T
        idx16_list.append(i16)
        dw_list.append(np.ascontiguousarray(dw))

    windows = []
    for w in range(n_w):
        groups = []
        for q in range(n_tab):
            for si in range(nsi_of_w[w]):
                k = int(Kmax[w, q, si])
                if k:
                    groups.append((q, si, k))
        windows.append((w, groups))

    gathers = []
    for w in range(n_w):
        for q in range(n_tab):
            lo = int(starts[w, q, 0])
            hi = lo + int(slots_per_group[w, q, :nsi_of_w[w]].sum())
            p = lo
            while p < hi:
                n = min(GCAP, hi - p)
                gathers.append((q, p, n))
                p += n
    return dict(idx16=idx16_list, dw=dw_list, windows=windows, gathers=gathers,
                total=total)


def _host_prep(cfg, inputs):
    f32 = np.float32
    x_src = np.asarray(inputs["x_src"], f32)
    x_dst = np.asarray(inputs["x_dst"], f32)
    src_idx = np.asarray(inputs["src_idx"]).astype(np.int64)
    dst_idx = np.asarray(inputs["dst_idx"]).astype(np.int64)
    L = np.asarray(inputs["W_msg_sd"]).shape[0]

    ns, nd = x_src.shape[0], x_dst.shape[0]
    xs = np.zeros((cfg.NS_PAD, D), f32)
    xs[:ns] = x_src
    xd = np.zeros((cfg.ND_PAD, D), f32)
    xd[:nd] = x_dst

    plan_sd = _build_plan(cfg, src_idx, dst_idx, _src_table_map, 4, cfg.DST_SH)
    plan_ds = _build_plan(cfg, dst_idx, src_idx, _dst_table_map, 2, cfg.SRC_SH)

    def stack_wb(wk, bk):
        w = np.asarray(inputs[wk], f32)
        b = np.asarray(inputs[bk], f32)
        out = np.empty((L, D + 1, D), np.float16)
        out[:, :D] = w.astype(np.float16)
        out[:, D] = b.astype(np.float16)
        return out

    host = dict(
        L=L,
        Wbm_sd=stack_wb("W_msg_sd", "b_msg_sd"),
        Wbm_ds=stack_wb("W_msg_ds", "b_msg_ds"),
        Wu_dst=np.asarray(inputs["W_upd_dst"], f32).astype(np.float16),
        Wu_src=np.asarray(inputs["W_upd_src"], f32).astype(np.float16),
        bu_dst=np.asarray(inputs["b_upd_dst"], f32)[:, :, None],
        bu_src=np.asarray(inputs["b_upd_src"], f32)[:, :, None],
        Win_src=np.asarray(inputs["W_in_src"], f32),
        Win_dst=np.asarray(inputs["W_in_dst"], f32),
        bin_src=np.asarray(inputs["b_in_src"], f32)[:, None],
        bin_dst=np.asarray(inputs["b_in_dst"], f32)[:, None],
        iota=np.tile(np.arange(WIN, dtype=np.float16), (128, 1)),
        xsT=[np.ascontiguousarray(xs[c * cfg.SRC_SH:(c + 1) * cfg.SRC_SH].T)
             for c in range(N_CORES)],
        xdT=[np.ascontiguousarray(xd[c * cfg.DST_SH:(c + 1) * cfg.DST_SH].T)
             for c in range(N_CORES)],
        plan_sd=plan_sd, plan_ds=plan_ds,
    )
    return host


def _build_nc(cfg, host):
    import concourse.bass as bass
    import concourse.tile as tile
    from concourse import bacc, mybir

    dt = mybir.dt
    L = host["L"]
    plan_sd, plan_ds = host["plan_sd"], host["plan_ds"]
    TOT_SD, TOT_DS = plan_sd["total"], plan_ds["total"]

    nc = bacc.Bacc("TRN2", target_bir_lowering=False, debug=False,
                   num_devices=N_CORES, num_swdge_queues=NQ)

    def inp(name, shape, dtype):
        return nc.dram_tensor(name, shape, dtype, kind="ExternalInput").ap()

    xT_src = inp("xT_src", [D, cfg.SRC_SH], dt.float32)
    xT_dst = inp("xT_dst", [D, cfg.DST_SH], dt.float32)
    Win_src = inp("Win_src", [D, D], dt.float32)
    Win_dst = inp("Win_dst", [D, D], dt.float32)
    bin_src = inp("bin_src", [D, 1], dt.float32)
    bin_dst = inp("bin_dst", [D, 1], dt.float32)
    Wbm_sd = inp("Wbm_sd", [L, D + 1, D], dt.float16)
    Wbm_ds = inp("Wbm_ds", [L, D + 1, D], dt.float16)
    Wu_dst = inp("Wu_dst", [L, 2 * D, D], dt.float16)
    Wu_src = inp("Wu_src", [L, 2 * D, D], dt.float16)
    bu_dst = inp("bu_dst", [L, D, 1], dt.float32)
    bu_src = inp("bu_src", [L, D, 1], dt.float32)
    iota_in = inp("iota", [128, WIN], dt.float16)
    idx_sd = inp("idx_sd", [128, TOT_SD // 16], dt.int16)
    dw_sd = inp("dw_sd", [128, TOT_SD // CHUNK], dt.float16)
    idx_ds = inp("idx_ds", [128, TOT_DS // 16], dt.int16)
    dw_ds = inp("dw_ds", [128, TOT_DS // CHUNK], dt.float16)
    out_hd = nc.dram_tensor("out_hd", [D, cfg.DST_SH], dt.float16,
                            kind="ExternalOutput").ap()

    # per-piece z shard (local) and AllGathered piece tensors (shared)
    zs_sh = [nc.dram_tensor(f"zs_sh{p}", [cfg.S_HALF, 128], dt.float16).ap()
             for p in range(2)]
    zd_sh = [nc.dram_tensor(f"zd_sh{p}", [cfg.D_HALF, 128], dt.float16).ap()
             for p in range(2)]
    zs_pc = [nc.dram_tensor(f"zs_pc{p}", [cfg.S_PIECE, 128], dt.float16,
                            addr_space="Shared").ap() for p in range(2)]
    zd_pc = [nc.dram_tensor(f"zd_pc{p}", [cfg.D_TAB, 128], dt.float16,
                            addr_space="Shared").ap() for p in range(2)]

    RELU = mybir.ActivationFunctionType.Relu
    EQ = mybir.AluOpType.is_equal
    rg = [list(range(N_CORES))]

    with tile.TileContext(nc) as tc:
        from contextlib import ExitStack
        with ExitStack() as ctx:
            pers = ctx.enter_context(tc.tile_pool(name="pers", bufs=1))
            ps_agg = ctx.enter_context(
                tc.tile_pool(name="psagg", bufs=4, space="PSUM"))
            ps_mlp = ctx.enter_context(
                tc.tile_pool(name="psmlp", bufs=3, space="PSUM"))
            gath = ctx.enter_context(tc.tile_pool(name="gath", bufs=10))
            idxg = ctx.enter_context(tc.tile_pool(name="idxg", bufs=16))
            spool = ctx.enter_context(tc.tile_pool(name="spool", bufs=4))
            work = ctx.enter_context(tc.tile_pool(name="work", bufs=4))

            h_s = pers.tile([D + 1, cfg.SRC_SH], dt.float16, name="h_s")
            h_d = pers.tile([D + 1, cfg.DST_SH], dt.float16, name="h_d")
            iota_t = pers.tile([128, WIN], dt.float16)
            dw_sd_t = pers.tile([128, TOT_SD // CHUNK], dt.float16)
            dw_ds_t = pers.tile([128, TOT_DS // CHUNK], dt.float16)

            nc.sync.dma_start(out=iota_t[:], in_=iota_in[:])
            nc.sync.dma_start(out=dw_sd_t[:], in_=dw_sd[:])
            nc.sync.dma_start(out=dw_ds_t[:], in_=dw_ds[:])

            w_enc_s = pers.tile([D, D], dt.float32)
            w_enc_d = pers.tile([D, D], dt.float32)
            b_enc_s = pers.tile([D, 1], dt.float32)
            b_enc_d = pers.tile([D, 1], dt.float32)
            nc.sync.dma_start(out=w_enc_s[:], in_=Win_src[:])
            nc.sync.dma_start(out=w_enc_d[:], in_=Win_dst[:])
            nc.sync.dma_start(out=b_enc_s[:], in_=bin_src[:])
            nc.sync.dma_start(out=b_enc_d[:], in_=bin_dst[:])

            wbm_t, wu_t, bu_t = {}, {}, {}
            for l in range(L):
                for key, src in (("sd", Wbm_sd), ("ds", Wbm_ds)):
                    t = pers.tile([D + 1, D], dt.float16, name=f"wbm_{key}{l}")
                    nc.sync.dma_start(out=t[:], in_=src[l])
                    wbm_t[key, l] = t
                for key, src in (("dst", Wu_dst), ("src", Wu_src)):
                    th = pers.tile([D, D], dt.float16, name=f"wuh_{key}{l}")
                    ta = pers.tile([D, D], dt.float16, name=f"wua_{key}{l}")
                    nc.sync.dma_start(out=th[:], in_=src[l, 0:D, :])
                    nc.sync.dma_start(out=ta[:], in_=src[l, D:2 * D, :])
                    wu_t[key, l] = (th, ta)
                for key, src in (("dst", bu_dst), ("src", bu_src)):
                    t = pers.tile([D, 1], dt.float32, name=f"bu_{key}{l}")
                    nc.sync.dma_start(out=t[:], in_=src[l])
                    bu_t[key, l] = t

            for t in (h_s, h_d):
                nc.vector.memset(t[D:D + 1, :], 1.0)

            # one-time zero fill of z-shard pad columns (never written later)
            zeros64 = pers.tile([128, D], dt.float16, name="zeros64")
            nc.vector.memset(zeros64[:], 0.0)
            for z_list, half in ((zs_sh, cfg.S_HALF), (zd_sh, cfg.D_HALF)):
                for z in z_list:
                    for k in range(0, half, CHUNK):
                        cw = min(CHUNK, half - k)
                        nc.sync.dma_start(out=z[k:k + cw, D:128],
                                          in_=zeros64[:cw, :])

            def encoder(xT, w_t, b_t, h_out, n, hooks=None):
                for j0 in range(0, n, WIN):
                    w = min(WIN, n - j0)
                    xs = work.tile([D, WIN], dt.float32, tag="xs")
                    nc.sync.dma_start(out=xs[:, :w], in_=xT[:, j0:j0 + w])
                    ps = ps_mlp.tile([D, WIN], dt.float32, tag="mlp")
                    nc.tensor.matmul(out=ps[:, :w], lhsT=w_t[:], rhs=xs[:, :w],
                                     start=True, stop=True)
                    nc.scalar.activation(out=h_out[0:D, j0:j0 + w],
                                         in_=ps[:, :w], func=RELU, bias=b_t[:])
                    if hooks and j0 // WIN in hooks:
                        hooks[j0 // WIN]()

            def z_piece(h_in, wbm, z_sh, half, p):
                r0 = p * half
                for k in range(0, half, CHUNK):
                    cw = min(CHUNK, half - k)
                    ps = ps_mlp.tile([CHUNK, D], dt.float32, tag="mlp")
                    nc.tensor.matmul(
                        out=ps[:cw, :],
                        lhsT=h_in[0:D + 1, r0 + k:r0 + k + cw],
                        rhs=wbm[:], start=True, stop=True)
                    zs = work.tile([CHUNK, D], dt.float16, tag="zstage")
                    nc.vector.tensor_scalar_max(out=zs[:cw, :], in0=ps[:cw, :],
                                                scalar1=0.0)
                    nc.sync.dma_start(out=z_sh[k:k + cw, 0:D], in_=zs[:cw, :])



            def z_ag(h_in, wbm, z_sh, z_pc, half, p):
                z_piece(h_in, wbm, z_sh[p], half, p)
                nc.gpsimd.collective_compute(
                    "AllGather", mybir.AluOpType.bypass, replica_groups=rg,
                    ins=[z_sh[p].opt()], outs=[z_pc[p].opt()])

            def sweep(plan, pieces, tab_of, idx_dram, dw_t, h_io, wu, bu,
                      shard, last=False, hooks=None):
                gathers = plan["gathers"]
                gtiles = []
                for g, (q, s0, nsl) in enumerate(gathers):
                    it = idxg.tile([128, nsl // 16], dt.int16, tag="idxg")
                    nc.sync.dma_start(
                        out=it[:], in_=idx_dram[:, s0 // 16:(s0 + nsl) // 16])
                    gt = gath.tile([128, nsl], dt.float16, tag="gt")
                    nc.gpsimd.dma_gather(
                        gt[:].rearrange("p (b e) -> p b e", e=128),
                        tab_of(q), it[:], nsl, nsl, 128, single_packet=False,
                        queue_num=g % NQ)
                    gtiles.append((s0, nsl, gt))

                gi = 0
                c = 0  # global chunk cursor
                s_tile = None
                for (w, groups) in plan["windows"]:
                    nvis = sum(k for (_, _, k) in groups)
                    w0 = w * WIN
                    ww = min(WIN, shard - w0)
                    ps = ps_agg.tile([D, WIN], dt.float32, tag="agg")
                    done = 0
                    for (q, si, kk) in groups:
                        for _ in range(kk):
                            if c % SB == 0:
                                nb = min(SB, dw_t.shape[1] - c)
                                s_tile = spool.tile([128, nb * SUB],
                                                    dt.float16, tag="s")
                                nc.vector.tensor_tensor(
                                    out=s_tile[:].rearrange(
                                        "p (b e) -> p b e", e=SUB),
                                    in0=dw_t[:, c:c + nb, None].to_broadcast(
                                        [128, nb, SUB]),
                                    in1=iota_t[:, 0:SUB][:, None, :]
                                    .to_broadcast([128, nb, SUB]),
                                    op=EQ)
                            s0, nsl, gt = gtiles[gi]
                            if c * CHUNK >= s0 + nsl:
                                gi += 1
                                s0, nsl, gt = gtiles[gi]
                            blk = (c * CHUNK - s0) // CHUNK
                            g3 = gt[:].rearrange("p (b e) -> p b e", e=128)
                            nc.tensor.matmul(
                                out=ps[:, si * SUB:(si + 1) * SUB],
                                lhsT=g3[:, blk, 0:D],
                                rhs=s_tile[:].rearrange(
                                    "p (b e) -> p b e", e=SUB)[:, c % SB, :],
                                start=(done == 0), stop=(done == nvis - 1),
                                skip_group_check=True)
                            done += 1
                            c += 1
                    # fused update for this window
                    ag = work.tile([D, WIN], dt.float16, tag="aggstage")
                    nc.vector.tensor_copy(out=ag[:, :ww], in_=ps[:, :ww])
                    psu = ps_mlp.tile([D, WIN], dt.float32, tag="mlp")
                    nc.tensor.matmul(out=psu[:, :ww], lhsT=wu[0][:],
                                     rhs=h_io[0:D, w0:w0 + ww],
                                     start=True, stop=False,
                                     skip_group_check=True)
                    nc.tensor.matmul(out=psu[:, :ww], lhsT=wu[1][:],
                                     rhs=ag[:, :ww],
                                     start=False, stop=True,
                                     skip_group_check=True)
                    nc.scalar.activation(out=h_io[0:D, w0:w0 + ww],
                                         in_=psu[:, :ww], func=RELU,
                                         bias=bu[:])
                    if last:
                        nc.sync.dma_start(out=out_hd[:, w0:w0 + ww],
                                          in_=h_io[0:D, w0:w0 + ww])
                    if hooks and w in hooks:
                        hooks[w]()

            def sd_tab(q):
                return zs_pc[q // 2][(q % 2) * cfg.S_TAB:
                                    (q % 2 + 1) * cfg.S_TAB, :]

            def ds_tab(q):
                return zd_pc[q]

            encoder(xT_src, w_enc_s, b_enc_s, h_s, cfg.SRC_SH)
            encoder(xT_dst, w_enc_d, b_enc_d, h_d, cfg.DST_SH)
            for l in range(L):
                for p in range(2):
                    z_ag(h_s, wbm_t["sd", l], zs_sh, zs_pc, cfg.S_HALF, p)
                sweep(plan_sd, zs_pc, sd_tab, idx_sd, dw_sd_t, h_d,
                      wu_t["dst", l], bu_t["dst", l], cfg.DST_SH,
                      last=(l == L - 1))
                if l == L - 1:
                    break
                for p in range(2):
                    z_ag(h_d, wbm_t["ds", l], zd_sh, zd_pc, cfg.D_HALF, p)
                sweep(plan_ds, zd_pc, ds_tab, idx_ds, dw_ds_t, h_s,
                      wu_t["src", l], bu_t["src", l], cfg.SRC_SH)

    nc.compile()
    return nc


def make_in_maps(cfg, host):
    shared = dict(
        Win_src=host["Win_src"], Win_dst=host["Win_dst"],
        bin_src=host["bin_src"], bin_dst=host["bin_dst"],
        Wbm_sd=host["Wbm_sd"], Wbm_ds=host["Wbm_ds"],
        Wu_dst=host["Wu_dst"], Wu_src=host["Wu_src"],
        bu_dst=host["bu_dst"], bu_src=host["bu_src"],
        iota=host["iota"],
    )
    maps = []
    for c in range(N_CORES):
        m = dict(shared)
        m["xT_src"] = host["xsT"][c]
        m["xT_dst"] = host["xdT"][c]
        m["idx_sd"] = host["plan_sd"]["idx16"][c]
        m["dw_sd"] = host["plan_sd"]["dw"][c]
        m["idx_ds"] = host["plan_ds"]["idx16"][c]
        m["dw_ds"] = host["plan_ds"]["dw"][c]
        maps.append(m)
    return maps


def kernel(**inputs) -> np.ndarray:
    cfg = REAL_CFG
    host = _host_prep(cfg, inputs)
    nc = _build_nc(cfg, host)
    from concourse.bass_utils import run_bass_kernel_spmd
    res = run_bass_kernel_spmd(nc, make_in_maps(cfg, host),
                               core_ids=list(range(N_CORES)))
    nd = np.asarray(inputs["x_dst"]).shape[0]
    out = np.concatenate([res.results[c]["out_hd"].T for c in range(N_CORES)],
                         axis=0)[:nd]
    return out.astype(np.float32)
